# revision 30
# baseline (speedup 1.0000x reference)
"""Trainium2 Bass kernel for nn_MaxAssigner2D (span=2 shifted channel-max pool).

Math (per image, zero-padded borders):
    m[h, w]   = max_c x[h, w, c]
    out[h, w] = max over (dh, dw) in S of m[h-dh, w-dw]   (0 outside bounds)
    S = {(0,0), (1,0), (0,1), (1,1), (2,0), (0,2), (2,2)}

Distribution: pure data parallel, 2 images per core across 8 NeuronCores.

Per-core layout: partition p <-> 4-row band of the image (128 bands x 4 rows
= 512 rows).  The channel-max plane m for each image lives in SBUF as
[128, 6*514] bf16: per partition 2 halo rows (rows 4p-2, 4p-1, zero for p=0)
followed by the band's 4 rows, each row stored as [2 zero pad cols | 512 data
cols].  With that layout every shift (dh, dw) of the 7-term max is a pure
free-dim AP offset, and the zero padding of the reference comes for free.

Stage 1 (channel max) is a DVE tensor_tensor max tree, not tensor_reduce:
TT consumes 2 fp32 elements/cycle vs reduce's 1 (and reduce pays a ~7-cycle
accumulator-reset bubble per output group).  Level 1 reads the fp32 x tile
as [p, w, 16]+[p, w, 16] and writes bf16; levels 2..5 run on bf16 where
16-element step-1 runs enable the DVE 2x perf mode.  bf16 (rel err ~2^-9)
is far inside the 2e-2 tolerance.

Pipeline per 128-row x-chunk (32 KB/partition contiguous DMA):
    nc.sync DMA in -> DVE TT tree (32ch -> 1) writing into the m tile.
Then per image half: partition-shifted SBUF->SBUF DMA fills halo rows
(same nc.sync ring; single-ring issue order == Tile schedule order, so the
DMA-completion wait values Tile computes for the shift ops can never
reference a later-issued transfer -- cross-ring setups all produced ~25us
phantom stalls), 12 DVE tensor_max ops accumulate the 7 shifts into a bf16
acc (split into inner rows / halo-dependent boundary rows, each group
stored separately so output writes drain early), and SWDGE (gpsimd) DMA
stores cast bf16 -> f32 on the way out.

Measured: ~204-206 us per uncontended core (all-core profiling showed 7/8
cores at 205.0-207.8 us in one launch; input stream ~360 GB/s = the per-NC
HBM cap; DVE ~150 us busy, off the critical path).  The structural floor is
~194 us of HBM traffic (67.1 MB read + 2.1 MB write @ 358 GB/s) plus
~8.7 us fixed Bacc preamble and ~2.4 us teardown.  Co-tenant HBM contention
adds 0-45 us on a wandering subset of cores, run to run.
"""

import numpy as np

import concourse.bacc as bacc
import concourse.bass as bass
import concourse.mybir as mybir
from concourse.tile import TileContext, add_dep_helper

F32 = mybir.dt.float32
BF16 = mybir.dt.bfloat16
NCORES = 8

# Full-problem geometry (hardcoded; kernel.py must be self-contained).
B, H, W, C = 16, 512, 512, 32
SPAN = 2


def build_nc(bpc, h, w, c, ph, qw):
    """Build the per-core Bass module.

    bpc: images per core; h/w/c: image dims; ph: rows per band (partitions =
    h // ph); qw: pixels per stage-1 chunk per partition.
    """
    P = h // ph               # partitions used
    assert P <= 128
    nq = w // qw              # chunks per band row
    rowp = SPAN + w           # padded row width (left zero pad only)
    mrows = ph + SPAN         # SPAN halo rows + band rows

    nc = bacc.Bacc("TRN2")
    x = nc.declare_dram_parameter("x", [bpc, h, w, c], F32, isOutput=False)
    out = nc.declare_dram_parameter("out", [bpc, h, w, 1], F32, isOutput=True)

    # DRAM views: partition p <-> band p
    xr = x.ap().rearrange("b (p ph) w c -> b p ph (w c)", ph=ph)     # [bpc,P,ph,w*c]
    outr = out.ap().rearrange("b (p ph) w c -> b p ph (w c)", ph=ph)  # [bpc,P,ph,w]

    with TileContext(nc) as tc:
        with (
            tc.tile_pool(name="xp", bufs=4) as xpool,
            tc.tile_pool(name="sp", bufs=2) as spool,
            tc.tile_pool(name="mp", bufs=1) as mpool,
            tc.tile_pool(name="op", bufs=2) as opool,
        ):
            # Persistent per-image m tiles (bf16).  Only the zero-pad regions
            # need clearing: the 2 left pad cols of every row, and the p=0
            # halo rows (all other halo rows are DMA-filled each half).
            m_tiles = [
                mpool.tile([P, mrows * rowp], BF16, tag=f"m{bi}", name=f"m{bi}")
                for bi in range(bpc)
            ]
            for mt in m_tiles:
                mt3_ = mt[:].rearrange("p (r w) -> p r w", w=rowp)
                nc.gpsimd.memset(mt3_[:, :, 0:SPAN], 0.0)
                nc.gpsimd.memset(mt3_[0:1, 0:SPAN, :], 0.0)

            # Symmetric column halves.  Keeping input DMAs at qw px/
            # partition (32 KB contiguous per partition) sustains the
            # ~360 GB/s stream -- narrower chunks measurably drop the line
            # rate (3-segment last-image split cost more stream time than
            # its smaller drain tail saved).
            def segs_for(bi):
                return [(0, w // 2), (w // 2, w)]

            def tree_reduce(xt, sw, dst):
                """Channel max 32 -> 1 of fp32 tile region xt ([P, sw*c]) into
                bf16 dst ([P, sw]) via a TT max tree with bf16 intermediates."""
                x3 = xt.rearrange("p (w c) -> p w c", c=c)          # [P,sw,32]
                # Fixed-size scratch slots (tag-uniform for the tile pool);
                # chunks narrower than qw just use a prefix.
                sc = spool.tile([P, qw * (c // 2 + c // 4 + c // 8 + c // 16)],
                                BF16, tag="sc", name="sc")
                a = sc[:, 0:sw * 16].rearrange("p (w c) -> p w c", c=16)
                b = sc[:, sw * 16:sw * 24].rearrange("p (w c) -> p w c", c=8)
                d = sc[:, sw * 24:sw * 28].rearrange("p (w c) -> p w c", c=4)
                e = sc[:, sw * 28:sw * 30].rearrange("p (w c) -> p w c", c=2)
                nc.vector.tensor_max(a, x3[:, :, 0:16], x3[:, :, 16:32])
                nc.vector.tensor_max(b, a[:, :, 0:8], a[:, :, 8:16])
                nc.vector.tensor_max(d, b[:, :, 0:4], b[:, :, 4:8])
                nc.vector.tensor_max(e, d[:, :, 0:2], d[:, :, 2:4])
                nc.vector.tensor_max(dst, e[:, :, 0], e[:, :, 1])

            for bi in range(bpc):
                mt = m_tiles[bi]
                mt3 = mt[:].rearrange("p (r w) -> p r w", w=rowp)  # [P,mrows,rowp]
                acc = opool.tile([P, ph * w], BF16, tag="acc", name="acc")
                a3 = acc[:].rearrange("p (r w) -> p r w", w=w)  # [P,ph,w]

                # Column segments per image: stream a segment's chunks, then
                # do its shifts + stores while the next segment streams.
                segs = segs_for(bi)
                for half, (w0, w1) in enumerate(segs):
                    hw = w1 - w0
                    # chunk column starts within [w0, w1), qw wide
                    qstarts = list(range(w0, w1, qw))
                    # ---- stage 1: channel max into m tile (TT tree) ----
                    # Halo source rows (ph-2, ph-1) first.  The very first
                    # chunk of the kernel and the very last are split into
                    # sub-chunks (pipeline fill / drain-tail shrink).
                    first_chunk = bi == 0 and half == 0
                    for r in (ph - 2, ph - 1, *range(ph - 2)):
                        last_chunk = (bi == bpc - 1 and half == len(segs) - 1
                                      and r == ph - 3)
                        for q0 in qstarts:
                            cw = min(qw, w1 - q0)
                            nsub = 4 if (first_chunk or last_chunk) else 1
                            first_chunk = False
                            sw = cw // nsub
                            for s in range(nsub):
                                p0 = q0 + s * sw
                                xt = xpool.tile(
                                    [P, qw * c], F32, tag="xt", name="xt"
                                )
                                nc.sync.dma_start(
                                    out=xt[:, 0:sw * c],
                                    in_=xr[bi, :, r, p0 * c:(p0 + sw) * c],
                                )
                                tree_reduce(
                                    xt[:, 0:sw * c], sw,
                                    mt3[:, SPAN + r, SPAN + p0:SPAN + p0 + sw],
                                )
                    # ---- stage 2: halo rows for this segment's columns,
                    # partition-shifted SBUF->SBUF copy on the SP ring.
                    # (A TensorE shift-matmul halo that would unload SDMA
                    # engine 64 -- the ring's long pole, see memory notes --
                    # produced inf outputs and was reverted; root cause not
                    # yet isolated.)
                    c0 = 0 if half == 0 else SPAN + w0
                    c1 = SPAN + w1 if half < len(segs) - 1 else rowp
                    nc.sync.dma_start(
                        out=mt3[1:P, 0:SPAN, c0:c1],
                        in_=mt3[0:P - 1, ph:ph + SPAN, c0:c1],
                    )

                    # ---- stage 3: 7-shift max for this half's outputs ----
                    # Ordered so every op that touches halo rows comes last:
                    # dh=0 / inner-row work overlaps the halo DMA's
                    # completion instead of idling on it.
                    def opnd(dh, dw, lo, hi):
                        # operand rows for out band rows [lo, hi)
                        return mt3[
                            :,
                            SPAN + lo - dh:SPAN + hi - dh,
                            SPAN - dw + w0:SPAN - dw + w0 + hw,
                        ]

                    # (An f32-acc + HWDGE-store variant for the final
                    # segment measured no better than this uniform SWDGE
                    # cast-store path, so all segments share it.)
                    dst3, dcol, store_eng = a3, w0, nc.gpsimd

                    def amax(lo, hi, dh, dw, first=False):
                        dst = dst3[:, lo:hi, dcol:dcol + hw]
                        src0 = opnd(0, 0, lo, hi) if first else dst
                        nc.vector.tensor_max(dst, src0, opnd(dh, dw, lo, hi))

                    # Inner rows (SPAN:ph) first: they never read halo rows
                    # and only depend on earlier-streamed chunks, so their
                    # 7-term max and store drain while later chunks stream.
                    # Boundary rows (0:SPAN) read the halo + the two last-
                    # streamed band rows; keeping their op set (and store)
                    # separate and small shrinks the post-stream tail.
                    amax(SPAN, ph, 0, 1, first=True)
                    amax(SPAN, ph, 0, 2)
                    for dh, dw in [(1, 0), (1, 1), (2, 0), (2, 2)]:
                        amax(SPAN, ph, dh, dw)
                    # ---- store inner rows ----
                    store_eng.dma_start(
                        out=outr[bi, :, SPAN:ph, w0:w0 + hw],
                        in_=dst3[:, SPAN:ph, dcol:dcol + hw],
                    )
                    amax(0, SPAN, 0, 1, first=True)
                    amax(0, SPAN, 0, 2)
                    for dh, dw in [(1, 0), (1, 1), (2, 0), (2, 2)]:
                        amax(0, SPAN, dh, dw)
                    # ---- store boundary rows ----
                    store_eng.dma_start(
                        out=outr[bi, :, 0:SPAN, w0:w0 + hw],
                        in_=dst3[:, 0:SPAN, dcol:dcol + hw],
                    )

    nc.finalize()
    return nc


_NC_CACHE = {}


def _get_nc():
    key = "full"
    if key not in _NC_CACHE:
        _NC_CACHE[key] = build_nc(B // NCORES, H, W, C, ph=4, qw=256)
    return _NC_CACHE[key]


def _run(x, trace=False):
    """Run the SPMD kernel on 8 cores. Returns (out, BassKernelResults)."""
    from concourse.bass_utils import run_bass_kernel_spmd

    x = np.ascontiguousarray(np.asarray(x), dtype=np.float32)
    assert x.shape == (B, H, W, C)
    bpc = B // NCORES
    nc = _get_nc()
    in_maps = [
        {"x": np.ascontiguousarray(x[i * bpc:(i + 1) * bpc])} for i in range(NCORES)
    ]
    res = run_bass_kernel_spmd(nc, in_maps, list(range(NCORES)), trace=trace)
    out = np.concatenate([res.results[i]["out"] for i in range(NCORES)], axis=0)
    return out, res


def kernel(x):
    out, _ = _run(x, trace=False)
    return out
